# revision 1
# baseline (speedup 1.0000x reference)
"""Trainium2 Bass kernel for CovClassifier (MPN-COV style).

Pipeline (per sample): covariance pooling -> Newton-Schulz matrix sqrt (5
iters) -> upper-triangle extraction fused with a 2-class FC.

Sharding: pure data parallel over the batch dim across 8 NeuronCores
(32 samples/core). The FC weight (scattered to a [2,256,256] upper-tri
matrix) is replicated.

Math notes:
- A = cov(x) is exactly symmetric; every Newton-Schulz iterate is a
  polynomial in A, hence symmetric and commuting. So for the PE's
  out = lhsT.T @ rhs we can pass the untransposed matrix as lhsT.
- trace(A) = sum_c var_c (biased var over the 196 spatial positions), which
  bn_stats/bn_aggr give directly.
- triuvec + FC is computed as <Y, Q_k> where Q_k is fc_w scattered into the
  upper triangle (host-precomputed); Y is used full (symmetric).
- Matrices are stored as [128, 512] tiles: col = mc*256 + j holds element
  (mc*128 + p, j) for partition p (two stacked 128-row blocks).
- All 256^3 matmuls run in float32r (~12-bit mantissa, full PE rate at
  N=256). Measured end-to-end logits error vs fp32 reference: ~1e-4.
- The final scale by sqrt(trace) and the bias add happen on the host
  (exactly commutes with the linear FC).
"""

import numpy as np

import concourse.bacc as bacc
import concourse.mybir as mybir
import concourse.tile as tile
from concourse.bass_utils import run_bass_kernel_spmd

dt = mybir.dt
ALU = mybir.AluOpType

B = 256
C = 256
HW = 196
NCORES = 8
NB = B // NCORES  # samples per core


def build(nb=NB, repeat=1, sim_safe=False):
    nc = bacc.Bacc("TRN2", target_bir_lowering=False, debug=False)

    x_d = nc.declare_dram_parameter("x", [nb, C, HW], dt.float32, isOutput=False)
    id_d = nc.declare_dram_parameter("id128", [128, 128], dt.float32, isOutput=False)
    i15_d = nc.declare_dram_parameter("i15", [128, 512], dt.float32, isOutput=False)
    q_d = nc.declare_dram_parameter("qmat", [128, 1024], dt.float32, isOutput=False)
    raw_d = nc.declare_dram_parameter("raw", [1, 2 * nb], dt.float32, isOutput=True)
    svar_d = nc.declare_dram_parameter("svar", [1, nb], dt.float32, isOutput=True)

    with tile.TileContext(nc) as tc:
        with (
            tc.tile_pool(name="consts", bufs=1) as cpool,
            tc.tile_pool(name="xin", bufs=6) as xpool,
            tc.tile_pool(name="stats", bufs=6) as spool,
            tc.tile_pool(name="xc", bufs=5) as xcpool,
            tc.tile_pool(name="mats", bufs=10) as mpool,
            tc.tile_pool(name="scr", bufs=4) as scrpool,
            tc.tile_pool(name="psmm", bufs=4, space="PSUM") as pmm,
            tc.tile_pool(name="psmm2", bufs=2, space="PSUM") as pmm2,
        ):
            # ---- constants ----
            id_sb = cpool.tile([128, 128], dt.float32, name="id_sb")
            nc.sync.dma_start(out=id_sb, in_=id_d[:, :])
            i15_sb = cpool.tile([128, 512], dt.float32, name="i15_sb")
            nc.sync.dma_start(out=i15_sb, in_=i15_d[:, :])
            q_sb = cpool.tile([128, 1024], dt.float32, name="q_sb")
            nc.sync.dma_start(out=q_sb, in_=q_d[:, :])
            ones_sb = cpool.tile([128, 128], dt.float32, name="ones_sb")
            nc.vector.memset(ones_sb, 1.0)
            acc_sb = cpool.tile([128, 2 * nb], dt.float32, name="acc_sb")
            svar_sb = cpool.tile([1, nb], dt.float32, name="svar_sb")

            def mm256(lhs, rhs):
                """psum[128,512] = lhs @ rhs for 256x256 symmetric operands in
                stacked-row-block layout (lhs passed as lhsT, valid since
                symmetric)."""
                ps = pmm.tile([128, 512], dt.float32, tag="mm", name="mmps")
                for cb in (0, 1):
                    for mc in (0, 1):
                        nc.tensor.matmul(
                            ps[:, cb * 256 : cb * 256 + 256],
                            lhs[:, mc * 256 + cb * 128 : mc * 256 + cb * 128 + 128],
                            rhs[:, mc * 256 : mc * 256 + 256],
                            start=(mc == 0),
                            stop=(mc == 1),
                        )
                return ps

            def step_load(b):
                x_sb = xpool.tile([128, 2, HW], dt.float32, tag="x", name="x_sb")
                for cb in (0, 1):
                    nc.sync.dma_start(
                        out=x_sb[:, cb, :], in_=x_d[b, cb * 128 : cb * 128 + 128, :]
                    )
                return x_sb

            def step_stats(x_sb):
                st = spool.tile([128, 2, 6], dt.float32, tag="st", name="st")
                mv = spool.tile([128, 2, 2], dt.float32, tag="mv", name="mv")
                for cb in (0, 1):
                    nc.vector.bn_stats(out=st[:, cb, :], in_=x_sb[:, cb, :])
                    nc.vector.bn_aggr(out=mv[:, cb, :], in_=st[:, cb, :])
                return mv

            def step_center(x_sb, mv):
                xc = xcpool.tile([128, 2, HW], dt.float32, tag="xc", name="xc")
                for cb in (0, 1):
                    nc.vector.tensor_scalar(
                        out=xc[:, cb, :],
                        in0=x_sb[:, cb, :],
                        scalar1=mv[:, cb, 0:1],
                        scalar2=None,
                        op0=ALU.subtract,
                    )
                return xc

            def step_transpose(xc):
                xt_ps = pmm.tile([128, 512], dt.float32, tag="mm", name="xt_ps")
                for mc in (0, 1):
                    msz = 128 if mc == 0 else HW - 128
                    for cb in (0, 1):
                        co = mc * 256 + cb * 128
                        nc.tensor.transpose(
                            xt_ps[0:msz, co : co + 128],
                            xc[:, cb, mc * 128 : mc * 128 + msz],
                            id_sb[:, :],
                        )
                return xt_ps

            def step_xt_copy(xt_ps):
                xt = mpool.tile([128, 512], dt.float32r, tag="xt", name="xt")
                if sim_safe:
                    nc.scalar.copy(out=xt[:, 0:256], in_=xt_ps[:, 0:256])
                    nc.scalar.copy(
                        out=xt[0 : HW - 128, 256:512],
                        in_=xt_ps[0 : HW - 128, 256:512],
                    )
                else:
                    # rows 68:128 of the right half are uninitialized psum;
                    # copied garbage is never read (cov uses rows 0:68 there)
                    nc.scalar.copy(out=xt, in_=xt_ps)
                return xt

            def step_cov(xt):
                g_ps = pmm.tile([128, 512], dt.float32, tag="mm", name="g_ps")
                for cb in (0, 1):
                    for mc in (0, 1):
                        msz = 128 if mc == 0 else HW - 128
                        co = mc * 256 + cb * 128
                        nc.tensor.matmul(
                            g_ps[:, cb * 256 : cb * 256 + 256],
                            xt[0:msz, co : co + 128],
                            xt[0:msz, mc * 256 : mc * 256 + 256],
                            start=(mc == 0),
                            stop=(mc == 1),
                        )
                return g_ps

            def stt_T(p_ps, tag="t"):
                t = mpool.tile([128, 512], dt.float32r, tag=tag, name=tag)
                nc.vector.scalar_tensor_tensor(
                    out=t,
                    in0=p_ps,
                    scalar=-0.5,
                    in1=i15_sb,
                    op0=ALU.mult,
                    op1=ALU.add,
                )
                return t

            def stt_T_from_sbuf(ahat):
                return stt_T(ahat, tag="z")

            def act_copy(ps, tag):
                m = mpool.tile([128, 512], dt.float32r, tag=tag, name=tag)
                nc.scalar.copy(out=m, in_=ps)
                return m

            GRP = 4
            groups, starts = [], []
            for _ in range(repeat):
                for gs in range(0, nb, GRP):
                    groups.append(list(range(gs, min(gs + GRP, nb))))
                    starts.append(gs)

            def prep_A(grp):
                return {"xs": [step_load(b) for b in grp], "grp": grp}

            def prep_B(st):
                st["mvs"] = [step_stats(x_sb) for x_sb in st["xs"]]
                st["xcs"] = [
                    step_center(x_sb, mv) for x_sb, mv in zip(st["xs"], st["mvs"])
                ]

            def prep_C(st, gs):
                gl = len(st["grp"])
                # trace-broadcast matmuls first; consume s_grp immediately so
                # its PSUM bank frees before the transposes need slots
                s_grp = pmm.tile([128, GRP], dt.float32, tag="mm", name="s_grp")
                for li in range(gl):
                    for cb in (0, 1):
                        nc.tensor.matmul(
                            s_grp[:, li : li + 1],
                            ones_sb[:, :],
                            st["mvs"][li][:, cb, 1:2],
                            start=(cb == 0),
                            stop=(cb == 1),
                        )
                recip = spool.tile([128, GRP], dt.float32, tag="recip", name="recip")
                nc.vector.reciprocal(out=recip[:, 0:gl], in_=s_grp[:, 0:gl])
                nc.scalar.copy(
                    out=svar_sb[0:1, gs : gs + gl], in_=s_grp[0:1, 0:gl]
                )
                recip196 = spool.tile(
                    [128, GRP], dt.float32, tag="recip196", name="r196"
                )
                nc.vector.tensor_scalar_mul(
                    recip196[:, 0:gl], recip[:, 0:gl], 1.0 / HW
                )
                st["recip196"] = recip196
                st["xt_pss"] = [step_transpose(xc) for xc in st["xcs"]]
                st["xts"] = [step_xt_copy(xt_ps) for xt_ps in st["xt_pss"]]

            def prep_D(st):
                gl = len(st["grp"])
                st["g_pss"] = [step_cov(xt) for xt in st["xts"]]
                recip196 = st["recip196"]
                ahats = []
                for li in range(gl):
                    ahat = mpool.tile(
                        [128, 512], dt.float32r, tag="ahat", name="ahat"
                    )
                    nc.scalar.mul(
                        out=ahat, in_=st["g_pss"][li], mul=recip196[:, li : li + 1]
                    )
                    ahats.append(ahat)
                st["ahats"] = ahats
                st["zs"] = [stt_T(ahat, tag="z") for ahat in ahats]

            def dve_pscopy(ps, tag):
                m = mpool.tile([128, 512], dt.float32r, tag=tag, name=tag)
                nc.vector.tensor_scalar_mul(m, ps, 1.0)
                return m

            def prep_E(st):
                gl = len(st["grp"])
                y_pss = [mm256(st["ahats"][li], st["zs"][li]) for li in range(gl)]
                st["ys"] = [act_copy(ps, "y") for ps in y_pss]

            def mm256_into(ps_slice, lhs, rhs):
                for cb in (0, 1):
                    for mc in (0, 1):
                        nc.tensor.matmul(
                            ps_slice[:, cb * 256 : cb * 256 + 256],
                            lhs[:, mc * 256 + cb * 128 : mc * 256 + cb * 128 + 128],
                            rhs[:, mc * 256 : mc * 256 + 256],
                            start=(mc == 0),
                            stop=(mc == 1),
                        )

            def ns_iter(st, it):
                gl = len(st["grp"])
                p_pss = [mm256(st["ys"][li], st["zs"][li]) for li in range(gl)]
                ts = [stt_T(ps) for ps in p_pss]
                nys, nzs = [], []
                for li in range(gl):
                    pair = pmm2.tile([128, 1024], dt.float32, tag="mm2", name="pair")
                    mm256_into(pair[:, 0:512], ts[li], st["ys"][li])
                    mm256_into(pair[:, 512:1024], ts[li], st["zs"][li])
                    yz = mpool.tile([128, 1024], dt.float32r, tag="yzp", name="yzp")
                    nc.scalar.copy(out=yz, in_=pair)
                    nys.append(yz[:, 0:512])
                    nzs.append(yz[:, 512:1024])
                st["ys"] = nys
                st["zs"] = nzs

            def ns_final(st):
                gl = len(st["grp"])
                p_pss = [mm256(st["ys"][li], st["zs"][li]) for li in range(gl)]
                ts = [stt_T(ps) for ps in p_pss]
                f_pss = [mm256(st["ys"][li], ts[li]) for li in range(gl)]
                for li in range(gl):
                    b = st["grp"][li]
                    for k in (0, 1):
                        scr = scrpool.tile(
                            [128, 512], dt.float32, tag="scr", name="scr"
                        )
                        nc.vector.scalar_tensor_tensor(
                            out=scr,
                            in0=f_pss[li],
                            scalar=1.0,
                            in1=q_sb[:, k * 512 : k * 512 + 512],
                            op0=ALU.mult,
                            op1=ALU.mult,
                            accum_out=acc_sb[:, 2 * b + k : 2 * b + k + 1],
                        )

            # 2-stage pipeline over groups: prep of group g+1 interleaves with
            # the Newton-Schulz chunks of group g.
            cur = prep_A(groups[0])
            prep_B(cur)
            prep_C(cur, starts[0])
            prep_D(cur)
            prep_E(cur)
            for g in range(len(groups)):
                nxt = None
                if g + 1 < len(groups):
                    nxt = prep_A(groups[g + 1])
                ns_iter(cur, 0)
                if nxt:
                    prep_B(nxt)
                ns_iter(cur, 1)
                if nxt:
                    prep_C(nxt, starts[g + 1])
                ns_iter(cur, 2)
                if nxt:
                    prep_D(nxt)
                ns_final(cur)
                if nxt:
                    prep_E(nxt)
                    cur = nxt

            # ---- cross-partition reduce of acc + writeback ----
            acc_ps = pmm.tile([1, 2 * nb], dt.float32, tag="mm", name="acc_ps")
            nc.tensor.matmul(
                acc_ps, ones_sb[:, 0:1], acc_sb[:, :], start=True, stop=True
            )
            raw_sb = cpool.tile([1, 2 * nb], dt.float32, name="raw_sb")
            nc.scalar.copy(out=raw_sb, in_=acc_ps)
            nc.sync.dma_start(out=raw_d[:, :], in_=raw_sb)
            nc.sync.dma_start(out=svar_d[:, :], in_=svar_sb)

    nc.compile()
    return nc


_CACHE = {}


def _host_consts(fc_w):
    """Build the host-side constant arrays."""
    id128 = np.eye(128, dtype=np.float32)
    i15 = np.zeros((128, 512), dtype=np.float32)
    i15[:, 0:128] = 1.5 * id128
    i15[:, 384:512] = 1.5 * id128
    # Q_k = scatter of fc_w row k into the upper triangle of [256,256]
    iu, ju = np.triu_indices(C)
    q = np.zeros((2, C, C), dtype=np.float32)
    q[:, iu, ju] = fc_w
    # device layout: q_sb[p, k*512 + mc*256 + j] = Q_k[mc*128+p, j]
    qh = np.zeros((128, 1024), dtype=np.float32)
    for k in range(2):
        for mc in range(2):
            qh[:, k * 512 + mc * 256 : k * 512 + mc * 256 + 256] = q[
                k, mc * 128 : mc * 128 + 128, :
            ]
    return id128, i15, qh


def kernel(x, fc_w, fc_b):
    x = np.ascontiguousarray(np.asarray(x, dtype=np.float32))
    fc_w = np.asarray(fc_w, dtype=np.float32)
    fc_b = np.asarray(fc_b, dtype=np.float32)

    xf = x.reshape(B, C, HW)
    id128, i15, qh = _host_consts(fc_w)

    if "nc" not in _CACHE:
        _CACHE["nc"] = build(NB)
    nc = _CACHE["nc"]

    in_maps = [
        {
            "x": np.ascontiguousarray(xf[i * NB : (i + 1) * NB]),
            "id128": id128,
            "i15": i15,
            "qmat": qh,
        }
        for i in range(NCORES)
    ]
    res = run_bass_kernel_spmd(nc, in_maps, list(range(NCORES)))

    out = np.empty((B, 2), dtype=np.float32)
    for i in range(NCORES):
        raw = res.results[i]["raw"].reshape(NB, 2)
        svar = res.results[i]["svar"].reshape(NB, 1)
        out[i * NB : (i + 1) * NB] = raw * np.sqrt(svar) + fc_b[None, :]
    return out

